# revision 1
# baseline (speedup 1.0000x reference)
"""CrossAttentionLayer kernel for 8x Trainium2 NeuronCores.

Problem (hardcoded): B=2, S=4096, HIDDEN=4096, HEADS=32, HEAD_DIM=128,
SLOTS=128, LN eps 1e-5.  out = x + (softmax(LN(x)@Wq.T split-heads @ K.T
/ sqrt(128), masked) @ V merge-heads) @ Wout.T

Strategy: data-parallel over the 8192 (B*S) rows — 1024 rows per core,
core c takes batch c//4.  Everything on-device except layout prep:
  * transposed dataflow: x.T [k, s] tiles; LN stats (mean/var over k =
    partition axis) via ones-matmul partition-broadcast sums; xn = bf16
  * Q-proj: QT[n,s] = (Wq*gamma).T-stationary @ xn.T, bias (beta@Wq.T)
    added on psum->sbuf copy; fused per-head with attention
  * attention per head in [t,s]/[d,s] layout: scoresT = K_h.T-st @ QT_h;
    exp on ACT (mask as per-partition bias, 1/sqrt(128) as scale);
    denominator via ones-matmul; attnT = V_h-st @ expT, normalized by
    reciprocal on the psum->sbuf copy (bf16)
  * out-proj: outT[n,s] = Wout.T-tiles-stationary @ attnT accumulated
    over heads; residual added from x.T f32; output outT per core,
    transposed/concatenated on host.
All matmuls bf16 x bf16 -> fp32 psum, N=512.
"""
import numpy as np
import ml_dtypes
import concourse.bass as bass
import concourse.mybir as mybir
import concourse.tile as tile
from concourse.vector_clock import ScopedClock

F32 = mybir.dt.float32
BF16 = mybir.dt.bfloat16
AF = mybir.ActivationFunctionType

B, S, HID, HEADS, DH, SLOTS = 2, 4096, 4096, 32, 128, 128
NC_ = 8
SC = B * S // NC_          # rows per core = 1024
KT = HID // 128            # 32 k-tiles
NT = HID // 128            # 32 n-tiles (= heads for Q)
NSL = SC // 512            # 2 moving slices of 512
EPS = 1e-5
SCALE = DH ** -0.5

_ws_counter = [0]


def _split_waits(nc, maxw=1):
    """This walrus build rejects >1 sync-wait per instruction: hoist
    extras into same-engine no-ops placed just before the instruction."""
    n = 0
    for f in nc.m.functions:
        for blk in f.blocks:
            insts = list(blk.instructions)
            out, dirty = [], False
            for inst in insts:
                si = inst.sync_info
                waits = list(si.on_wait) if (si is not None and si.on_wait) else []
                if len(waits) > maxw:
                    ups = list(si.on_update or [])
                    for i in range(maxw, len(waits), maxw):
                        _ws_counter[0] += 1
                        nop = mybir.InstNoOp(
                            name=f"I-ws{_ws_counter[0]}", ins=[], outs=[]
                        )
                        nop.engine = inst.engine
                        nop.sync_info = mybir.SyncInfo(
                            on_wait=waits[i : i + maxw], on_update=[]
                        )
                        out.append(nop)
                        n += 1
                    inst.sync_info = mybir.SyncInfo(
                        on_wait=waits[:maxw], on_update=ups
                    )
                    dirty = True
                out.append(inst)
            if dirty:
                blk.instructions = out
    return n


def _patch_tile_drain():
    import concourse.tile as tile_mod

    def _patched(self, tick_clock, wait_clock):
        nc = self.nc
        drain_inst = nc.sync.drain()
        wait_clock.add_sem_waits(
            drain_inst.ins, ScopedClock({None: tick_clock.global_clock})
        )
        inst = drain_inst.ins
        si = inst.sync_info
        waits = list(si.on_wait or []) if si is not None else []
        if len(waits) > 1:
            ups = list(si.on_update or []) if si is not None else []
            inst.sync_info = mybir.SyncInfo(on_wait=waits[:1], on_update=ups)
            for i in range(1, len(waits)):
                nop = nc.sync.nop()
                nop.ins.sync_info = mybir.SyncInfo(
                    on_wait=waits[i : i + 1], on_update=[]
                )
        nc.all_engine_barrier()
        assert self.sems is not None
        popped = nc._tile_sem_poison_stack.pop()
        assert popped is self._sem_poison
        nc.clear_and_free_semaphores(list(self.sems.allocated().values()))
        nc.all_engine_barrier()

    tile_mod.TileContext._drain_and_barrier = _patched


def build_nc():
    _patch_tile_drain()
    nc = bass.Bass()

    xtb_in = nc.dram_tensor("xtb", [HID, SC], BF16, kind="ExternalInput")
    xtf_in = nc.dram_tensor("xtf", [HID, SC], F32, kind="ExternalInput")
    wqt_in = nc.dram_tensor("wqt", [HID, HID], BF16, kind="ExternalInput")
    bq_in = nc.dram_tensor("bq", [128, NT], F32, kind="ExternalInput")
    wot_in = nc.dram_tensor("wot", [HID, HID], BF16, kind="ExternalInput")
    kt_in = nc.dram_tensor("ktt", [HEADS, DH, SLOTS], BF16, kind="ExternalInput")
    v_in = nc.dram_tensor("vv", [HEADS, SLOTS, DH], BF16, kind="ExternalInput")
    mb_in = nc.dram_tensor("mb", [SLOTS, 1], F32, kind="ExternalInput")
    out_t = nc.dram_tensor("outt", [HID, SC], F32, kind="ExternalOutput")

    with tile.TileContext(nc) as tc:
        with tc.tile_pool(name="persist", bufs=1) as P:
            ones = P.tile([128, 128], BF16, tag="ones")
            nc.vector.memset(ones[:], 1.0)
            eps_t = P.tile([128, 1], F32, tag="eps")
            nc.vector.memset(eps_t[:], EPS)
            kt_all = P.tile([128, HEADS, SLOTS], BF16, tag="kt")
            nc.sync.dma_start(
                kt_all[:], kt_in[:].rearrange("h d t -> d h t")
            )
            v_all = P.tile([128, HEADS, DH], BF16, tag="v")
            nc.sync.dma_start(v_all[:], v_in[:].rearrange("h t d -> t h d"))
            mb = P.tile([128, 1], F32, tag="mb")
            nc.sync.dma_start(mb[:], mb_in[:])
            bq = P.tile([128, NT], F32, tag="bq")
            nc.sync.dma_start(bq[:], bq_in[:])

            rstd_b = P.tile([128, SC], F32, tag="rstd")
            mrs_b = P.tile([128, SC], F32, tag="mrs")
            xn = [P.tile([128, SC], BF16, tag=f"xn{k}", name=f"xn{k}") for k in range(KT)]
            att = [P.tile([128, SC], BF16, tag=f"att{h}", name=f"att{h}") for h in range(HEADS)]

            # ---------- phase 1: LN stats ----------
            with (
                tc.tile_pool(name="xs", bufs=6) as XS,
                tc.tile_pool(name="sqp", bufs=4) as SQ,
                tc.tile_pool(name="stps", bufs=1, space="PSUM") as STP,
                tc.tile_pool(name="stsb", bufs=2) as STS,
            ):
                sum_ps = [STP.tile([128, 512], F32, tag=f"sum{sl}", name=f"sum{sl}") for sl in range(NSL)]
                ssq_ps = [STP.tile([128, 512], F32, tag=f"ssq{sl}", name=f"ssq{sl}") for sl in range(NSL)]
                for k in range(KT):
                    xt = XS.tile([128, SC], BF16, tag="xt")
                    nc.sync.dma_start(xt[:], xtb_in[k * 128 : (k + 1) * 128, :])
                    sq = SQ.tile([128, SC], BF16, tag="sq")
                    nc.scalar.square(sq[:], xt[:])
                    for sl in range(NSL):
                        cs = slice(sl * 512, (sl + 1) * 512)
                        nc.tensor.matmul(
                            sum_ps[sl][:], ones[:], xt[:, cs],
                            start=(k == 0), stop=(k == KT - 1),
                        )
                        nc.tensor.matmul(
                            ssq_ps[sl][:], ones[:], sq[:, cs],
                            start=(k == 0), stop=(k == KT - 1),
                        )
                for sl in range(NSL):
                    cs = slice(sl * 512, (sl + 1) * 512)
                    mean = STS.tile([128, 512], F32, tag="mean")
                    nc.vector.tensor_scalar_mul(mean[:], sum_ps[sl][:], 1.0 / HID)
                    esq = STS.tile([128, 512], F32, tag="esq")
                    nc.vector.tensor_scalar_mul(esq[:], ssq_ps[sl][:], 1.0 / HID)
                    msq = STS.tile([128, 512], F32, tag="msq")
                    nc.vector.tensor_mul(msq[:], mean[:], mean[:])
                    var = STS.tile([128, 512], F32, tag="var")
                    nc.vector.tensor_sub(var[:], esq[:], msq[:])
                    std = STS.tile([128, 512], F32, tag="std")
                    nc.scalar.activation(std[:], var[:], AF.Sqrt, bias=eps_t[:])
                    nc.vector.reciprocal(rstd_b[:, cs], std[:])
                    nc.vector.tensor_mul(mrs_b[:, cs], mean[:], rstd_b[:, cs])

            # ---------- phase 2: xn ----------
            with (
                tc.tile_pool(name="xs2", bufs=6) as XS2,
                tc.tile_pool(name="tmpp", bufs=4) as TMP,
            ):
                for k in range(KT):
                    xt = XS2.tile([128, SC], BF16, tag="xt2")
                    nc.sync.dma_start(xt[:], xtb_in[k * 128 : (k + 1) * 128, :])
                    tmp = TMP.tile([128, SC], F32, tag="tmp")
                    nc.vector.tensor_mul(tmp[:], xt[:], rstd_b[:])
                    nc.vector.tensor_sub(xn[k][:], tmp[:], mrs_b[:])

            # ---------- phase 3: per-head Q-proj + attention ----------
            with (
                tc.tile_pool(name="wq", bufs=2) as WQ,
                tc.tile_pool(name="qps", bufs=1, space="PSUM") as QPS,
                tc.tile_pool(name="qsb", bufs=2) as QSB,
                tc.tile_pool(name="aps", bufs=1, space="PSUM") as APS,
                tc.tile_pool(name="expp", bufs=2) as EXP,
                tc.tile_pool(name="rcp", bufs=2) as RCP,
            ):
                for h in range(HEADS):
                    wq = WQ.tile([128, KT, 128], BF16, tag="wq")
                    nc.sync.dma_start(
                        wq[:],
                        wqt_in[:, h * 128 : (h + 1) * 128].rearrange(
                            "(kt p) n -> p kt n", p=128
                        ),
                    )
                    qt_ps = [QPS.tile([128, 512], F32, tag=f"qt{sl}", name=f"qtp{sl}") for sl in range(NSL)]
                    for k in range(KT):
                        for sl in range(NSL):
                            cs = slice(sl * 512, (sl + 1) * 512)
                            nc.tensor.matmul(
                                qt_ps[sl][:], wq[:, k, :], xn[k][:, cs],
                                start=(k == 0), stop=(k == KT - 1),
                            )
                    qt = QSB.tile([128, SC], BF16, tag="qt")
                    for sl in range(NSL):
                        cs = slice(sl * 512, (sl + 1) * 512)
                        nc.vector.tensor_scalar_add(
                            qt[:, cs], qt_ps[sl][:], bq[:, h : h + 1]
                        )
                    expt = EXP.tile([128, SC], BF16, tag="expt")
                    for sl in range(NSL):
                        cs = slice(sl * 512, (sl + 1) * 512)
                        sc_ps = APS.tile([128, 512], F32, tag=f"sc{sl}")
                        nc.tensor.matmul(
                            sc_ps[:], kt_all[:, h, :], qt[:, cs],
                            start=True, stop=True,
                        )
                        nc.scalar.activation(
                            expt[:, cs], sc_ps[:], AF.Exp,
                            bias=mb[:], scale=SCALE,
                        )
                    for sl in range(NSL):
                        cs = slice(sl * 512, (sl + 1) * 512)
                        den_ps = APS.tile([128, 512], F32, tag=f"den{sl}")
                        nc.tensor.matmul(
                            den_ps[:], ones[:], expt[:, cs], start=True, stop=True
                        )
                        rcp = RCP.tile([128, 512], F32, tag="rcp")
                        nc.vector.reciprocal(rcp[:], den_ps[:])
                        at_ps = APS.tile([128, 512], F32, tag=f"at{sl}")
                        nc.tensor.matmul(
                            at_ps[:], v_all[:, h, :], expt[:, cs],
                            start=True, stop=True,
                        )
                        nc.vector.tensor_mul(att[h][:, cs], at_ps[:], rcp[:])

            # ---------- phase 4: out-proj + residual ----------
            with (
                tc.tile_pool(name="wo", bufs=2) as WO,
                tc.tile_pool(name="ops", bufs=2, space="PSUM") as OPS,
                tc.tile_pool(name="xrs", bufs=2) as XRS,
                tc.tile_pool(name="osb", bufs=3) as OSB,
            ):
                for nt in range(NT):
                    wo = WO.tile([128, KT, 128], BF16, tag="wo")
                    nc.sync.dma_start(
                        wo[:],
                        wot_in[:, nt * 128 : (nt + 1) * 128].rearrange(
                            "(ht p) n -> p ht n", p=128
                        ),
                    )
                    xr = XRS.tile([128, SC], F32, tag="xr")
                    nc.sync.dma_start(xr[:], xtf_in[nt * 128 : (nt + 1) * 128, :])
                    o_ps = [OPS.tile([128, 512], F32, tag=f"o{sl}", name=f"op{sl}") for sl in range(NSL)]
                    for h in range(HEADS):
                        for sl in range(NSL):
                            cs = slice(sl * 512, (sl + 1) * 512)
                            nc.tensor.matmul(
                                o_ps[sl][:], wo[:, h, :], att[h][:, cs],
                                start=(h == 0), stop=(h == HEADS - 1),
                            )
                    for sl in range(NSL):
                        cs = slice(sl * 512, (sl + 1) * 512)
                        osb = OSB.tile([128, 512], F32, tag="osb")
                        nc.vector.tensor_add(osb[:], o_ps[sl][:], xr[:, cs])
                        nc.sync.dma_start(
                            out_t[nt * 128 : (nt + 1) * 128, cs], osb[:]
                        )

    _split_waits(nc)
    return nc


_NC_CACHE = None
_LAST_IN_MAPS = None


def kernel(
    hidden_states, memory_keys, memory_values, attention_mask, Wq, Wout,
    ln_gamma, ln_beta,
):
    global _NC_CACHE
    if _NC_CACHE is None:
        _NC_CACHE = build_nc()
    nc = _NC_CACHE

    f32 = np.float32
    bf16 = ml_dtypes.bfloat16
    x = np.asarray(hidden_states, dtype=f32).reshape(B * S, HID)
    gamma = np.asarray(ln_gamma, dtype=f32)
    beta = np.asarray(ln_beta, dtype=f32)
    Wq = np.asarray(Wq, dtype=f32)
    Wout = np.asarray(Wout, dtype=f32)

    wqt = np.ascontiguousarray((Wq * gamma[None, :]).T).astype(bf16)
    bq = np.ascontiguousarray((Wq @ beta).reshape(NT, 128).T).astype(f32)
    wot = np.ascontiguousarray(Wout.T).astype(bf16)

    kts, vs, mbs = [], [], []
    for b in range(B):
        kb = np.asarray(memory_keys[b], dtype=f32).reshape(SLOTS, HEADS, DH)
        vb = np.asarray(memory_values[b], dtype=f32).reshape(SLOTS, HEADS, DH)
        kts.append(np.ascontiguousarray(kb.transpose(1, 2, 0)).astype(bf16))
        vs.append(np.ascontiguousarray(vb.transpose(1, 0, 2)).astype(bf16))
        m = np.asarray(attention_mask[b]).astype(bool)
        mbs.append(np.where(m, 0.0, -1e30).astype(f32).reshape(SLOTS, 1))

    in_maps = []
    for c in range(NC_):
        rows = slice(c * SC, (c + 1) * SC)
        xt = np.ascontiguousarray(x[rows].T)  # [HID, SC] f32
        b = (c * SC) // S
        in_maps.append(
            dict(
                xtb=xt.astype(bf16),
                xtf=xt,
                wqt=wqt,
                bq=bq,
                wot=wot,
                ktt=kts[b],
                vv=vs[b],
                mb=mbs[b],
            )
        )

    global _LAST_IN_MAPS
    _LAST_IN_MAPS = in_maps
    from concourse import bass2jax

    results = bass2jax.run_bass_via_pjrt(nc, in_maps, n_cores=NC_)

    out = np.empty((B * S, HID), dtype=f32)
    for c in range(NC_):
        out[c * SC : (c + 1) * SC] = results[c]["outt"].T
    return out.reshape(B, S, HID)



# revision 8
# speedup vs baseline: 1.4332x; 1.4332x over previous
"""CrossAttentionLayer kernel for 8x Trainium2 NeuronCores (fp8 edition).

Problem (hardcoded): B=2, S=4096, HIDDEN=4096, HEADS=32, HEAD_DIM=128,
SLOTS=128, LN eps 1e-5.  out = x + (softmax(LN(x)@Wq.T split-heads @ K.T
/ sqrt(128), masked) @ V merge-heads) @ Wout.T

Strategy: data-parallel over the 8192 (B*S) rows — 1024 rows per core,
core c takes batch c//4.  Transposed dataflow ([feature, token] tiles).

fp8 design (rel err ~7e-3 vs 2e-2 budget):
  * x.T quantized to fp8e4m3 on host; weights quantized as 64*(Wq*gamma).T
    and 64*Wout.T (values ~N(0,1): safely inside e4m3 normal range).
  * Both projections run fp8 DoubleRow matmuls: moving/stationary carry a
    [128, 2, *] pair of k-tiles, one instruction contracts 256 deep -> 2x.
  * LayerNorm is folded into the Q projection epilogue:
      Q = rstd/64 * ( Wq8.T @ x8  - csum8 (x) mean )        (rank-1 corr.)
    with csum8[n] = sum_k wq8[k,n] -- the correction rides as one K=1 bf16
    matmul into the same psum accumulation group.  Q bias (Wq@beta) is
    folded into the exp() per-partition bias on host:
      mbq[t,h] = mask_bias[t] + scale * (K_h @ bq_h)[t].
  * LN stats come from x8: sum via fp8-DoubleRow ones-matmul, sum-sq via
    fp8 squares (scalar engine) + DoubleRow ones-matmul.
  * attention math per head in [t,s]/[d,s] layout, bf16 (cheap: SLOTS=128).
    denominators via ones-matmul; reciprocal_approx_fast (5x faster DVE op).
  * att stored fp8 [128, HEADS, SC]; out-proj accumulates fp8 DoubleRow over
    head pairs; epilogue fuses (psum*1/64 + residual) in one DVE
    scalar_tensor_tensor; residual x.T streamed f32 from DRAM.
"""
import numpy as np
import ml_dtypes
import concourse.bass as bass
import concourse.mybir as mybir
import concourse.tile as tile
from concourse.vector_clock import ScopedClock

F32 = mybir.dt.float32
BF16 = mybir.dt.bfloat16
F8 = mybir.dt.float8e4
AF = mybir.ActivationFunctionType
ALU = mybir.AluOpType
DR = mybir.MatmulPerfMode.DoubleRow

B, S, HID, HEADS, DH, SLOTS = 2, 4096, 4096, 32, 128, 128
NC_ = 8
SC = B * S // NC_          # rows per core = 1024
KT = HID // 128            # 32 k-tiles
KP = KT // 2               # 16 k-tile pairs (DoubleRow)
NT = HID // 128            # 32 n-tiles (= heads for Q)
NSL = SC // 512            # 2 moving slices of 512
NCH = 4                    # x8 DMA chunks
KCH = KT // NCH            # 8 k-tiles per chunk
EPS = 1e-5
SCALE = DH ** -0.5
WS = 64.0                  # fp8 weight pre-scale

_ws_counter = [0]


def _split_waits(nc, maxw=1):
    """This walrus build rejects >1 sync-wait per instruction: hoist
    extras into same-engine no-ops placed just before the instruction."""
    n = 0
    for f in nc.m.functions:
        for blk in f.blocks:
            insts = list(blk.instructions)
            out, dirty = [], False
            for inst in insts:
                si = inst.sync_info
                waits = list(si.on_wait) if (si is not None and si.on_wait) else []
                if len(waits) > maxw:
                    ups = list(si.on_update or [])
                    for i in range(maxw, len(waits), maxw):
                        _ws_counter[0] += 1
                        nop = mybir.InstNoOp(
                            name=f"I-ws{_ws_counter[0]}", ins=[], outs=[]
                        )
                        nop.engine = inst.engine
                        nop.sync_info = mybir.SyncInfo(
                            on_wait=waits[i : i + maxw], on_update=[]
                        )
                        out.append(nop)
                        n += 1
                    inst.sync_info = mybir.SyncInfo(
                        on_wait=waits[:maxw], on_update=ups
                    )
                    dirty = True
                out.append(inst)
            if dirty:
                blk.instructions = out
    return n


def _patch_tile_drain():
    import concourse.tile as tile_mod

    def _patched(self, tick_clock, wait_clock):
        nc = self.nc
        drain_inst = nc.sync.drain()
        wait_clock.add_sem_waits(
            drain_inst.ins, ScopedClock({None: tick_clock.global_clock})
        )
        inst = drain_inst.ins
        si = inst.sync_info
        waits = list(si.on_wait or []) if si is not None else []
        if len(waits) > 1:
            ups = list(si.on_update or []) if si is not None else []
            inst.sync_info = mybir.SyncInfo(on_wait=waits[:1], on_update=ups)
            for i in range(1, len(waits)):
                nop = nc.sync.nop()
                nop.ins.sync_info = mybir.SyncInfo(
                    on_wait=waits[i : i + 1], on_update=[]
                )
        nc.all_engine_barrier()
        assert self.sems is not None
        popped = nc._tile_sem_poison_stack.pop()
        assert popped is self._sem_poison
        nc.clear_and_free_semaphores(list(self.sems.allocated().values()))
        nc.all_engine_barrier()

    tile_mod.TileContext._drain_and_barrier = _patched


def build_nc(split_waits=True):
    _patch_tile_drain()
    nc = bass.Bass()

    xt8_in = nc.dram_tensor("xt8", [HID, SC], F8, kind="ExternalInput")
    xtf_in = nc.dram_tensor("xtf", [HID, SC], F32, kind="ExternalInput")
    wqt_in = nc.dram_tensor("wqt", [HID, HID], F8, kind="ExternalInput")
    wot_in = nc.dram_tensor("wot", [HID, HID], F8, kind="ExternalInput")
    csn_in = nc.dram_tensor("csn", [1, HID], BF16, kind="ExternalInput")
    kt_in = nc.dram_tensor("ktt", [HEADS, DH, SLOTS], BF16, kind="ExternalInput")
    v_in = nc.dram_tensor("vv", [HEADS, SLOTS, DH], BF16, kind="ExternalInput")
    mbq_in = nc.dram_tensor("mbq", [SLOTS, HEADS], F32, kind="ExternalInput")
    out_t = nc.dram_tensor("outt", [HID, SC], F32, kind="ExternalOutput")

    with tile.TileContext(nc) as tc:
        with tc.tile_pool(name="persist", bufs=1) as P:
            ones8 = P.tile([128, 2, 128], F8, tag="ones8")
            nc.vector.memset(ones8[:], 1.0)
            ones16 = P.tile([128, 128], BF16, tag="ones16")
            nc.vector.memset(ones16[:], 1.0 / 16.0)
            eps_t = P.tile([128, 1], F32, tag="eps")
            nc.vector.memset(eps_t[:], EPS * HID)
            x8c = []
            for i in range(NCH):
                t = P.tile([128, KCH, SC], F8, tag=f"x8c{i}", name=f"x8c{i}")
                nc.sync.dma_start(
                    t[:],
                    xt8_in[i * KCH * 128 : (i + 1) * KCH * 128, :].rearrange(
                        "(kt p) s -> p kt s", p=128
                    ),
                )
                x8c.append(t)
            kt_all = P.tile([128, HEADS, SLOTS], BF16, tag="kt")
            nc.sync.dma_start(
                kt_all[:], kt_in[:].rearrange("h d t -> d h t")
            )
            v_all = P.tile([128, HEADS, DH], BF16, tag="v")
            nc.sync.dma_start(v_all[:], v_in[:].rearrange("h t d -> t h d"))
            mbq = P.tile([128, HEADS], F32, tag="mbq")
            nc.sync.dma_start(mbq[:], mbq_in[:])
            csn = P.tile([1, HID], BF16, tag="csn")
            nc.sync.dma_start(csn[:], csn_in[:])

            cr = P.tile([1, SC], BF16, tag="cr")          # mean row
            rstd64 = P.tile([128, SC], F32, tag="rstd")   # rstd/64 bcast
            att8 = P.tile([128, HEADS, SC], F8, tag="att8")

            # ---------- phase 1: LN stats from x8 ----------
            with (
                tc.tile_pool(name="sqp", bufs=3) as SQ,
                tc.tile_pool(name="stps", bufs=1, space="PSUM") as STP,
                tc.tile_pool(name="stsb", bufs=2) as STS,
            ):
                sum_ps = [STP.tile([128, 512], F32, tag=f"sum{sl}", name=f"sum{sl}") for sl in range(NSL)]
                ssq_ps = [STP.tile([128, 512], F32, tag=f"ssq{sl}", name=f"ssq{sl}") for sl in range(NSL)]
                for kp in range(KP):
                    xs = x8c[kp // (KCH // 2)][:, (2 * kp) % KCH : (2 * kp) % KCH + 2, :]
                    sq = SQ.tile([128, 2, SC], F8, tag="sq")
                    nc.scalar.square(sq[:], xs)
                    for sl in range(NSL):
                        cs = slice(sl * 512, (sl + 1) * 512)
                        nc.tensor.matmul(
                            sum_ps[sl][:], ones8[:], xs[:, :, cs],
                            start=(kp == 0), stop=(kp == KP - 1), perf_mode=DR,
                        )
                        nc.tensor.matmul(
                            ssq_ps[sl][:], ones8[:], sq[:, :, cs],
                            start=(kp == 0), stop=(kp == KP - 1), perf_mode=DR,
                        )
                for sl in range(NSL):
                    cs = slice(sl * 512, (sl + 1) * 512)
                    nc.vector.tensor_scalar_mul(cr[0:1, cs], sum_ps[sl][0:1, :], 1.0 / HID)
                    mean = STS.tile([128, 512], F32, tag="mean")
                    nc.vector.tensor_scalar_mul(mean[:], sum_ps[sl][:], 1.0 / HID)
                    esq = STS.tile([128, 512], F32, tag="esq")
                    nc.vector.tensor_scalar_mul(esq[:], ssq_ps[sl][:], 1.0 / HID)
                    var = STS.tile([128, 512], F32, tag="var")
                    # var = esq - mean*mean  ==  (mean * -mean?) ; use mul+sub
                    msq = STS.tile([128, 512], F32, tag="msq")
                    nc.vector.tensor_mul(msq[:], mean[:], mean[:])
                    nc.vector.tensor_sub(var[:], esq[:], msq[:])
                    std64 = STS.tile([128, 512], F32, tag="std64")
                    # 64*sqrt(var+eps) = sqrt(4096*var + 4096*eps)
                    nc.scalar.activation(std64[:], var[:], AF.Sqrt, bias=eps_t[:], scale=float(HID))
                    nc.vector.reciprocal(rstd64[:, cs], std64[:])

            # ---------- phase 3: per-head Q-proj + attention ----------
            with (
                tc.tile_pool(name="wq", bufs=2) as WQ,
                tc.tile_pool(name="qps", bufs=2, space="PSUM") as QPS,
                tc.tile_pool(name="qsb", bufs=2) as QSB,
                tc.tile_pool(name="aps", bufs=1, space="PSUM") as APS,
                tc.tile_pool(name="expp", bufs=2) as EXP,
                tc.tile_pool(name="prb", bufs=2) as PRB,
                tc.tile_pool(name="rcp", bufs=2) as RCP,
            ):
                for h in range(HEADS):
                    wq = WQ.tile([128, KT, 128], F8, tag="wq")
                    nc.sync.dma_start(
                        wq[:],
                        wqt_in[:, h * 128 : (h + 1) * 128].rearrange(
                            "(kt p) n -> p kt n", p=128
                        ),
                    )
                    qt_ps = [QPS.tile([128, 512], F32, tag=f"qt{sl}", name=f"qtp{sl}") for sl in range(NSL)]
                    for sl in range(NSL):
                        cs = slice(sl * 512, (sl + 1) * 512)
                        for kp in range(KP):
                            xs = x8c[kp // (KCH // 2)][:, (2 * kp) % KCH : (2 * kp) % KCH + 2, cs]
                            nc.tensor.matmul(
                                qt_ps[sl][:], wq[:, 2 * kp : 2 * kp + 2, :], xs,
                                start=(kp == 0), stop=False, perf_mode=DR,
                            )
                        nc.tensor.matmul(
                            qt_ps[sl][:], csn[0:1, h * 128 : (h + 1) * 128],
                            cr[0:1, cs], start=False, stop=True,
                        )
                    qt = QSB.tile([128, SC], BF16, tag="qt")
                    for sl in range(NSL):
                        cs = slice(sl * 512, (sl + 1) * 512)
                        nc.vector.tensor_mul(qt[:, cs], qt_ps[sl][:], rstd64[:, cs])
                    expt = EXP.tile([128, SC], BF16, tag="expt")
                    pr8 = PRB.tile([128, SC], F8, tag="pr8")
                    m_ps = []
                    for sl in range(NSL):
                        cs = slice(sl * 512, (sl + 1) * 512)
                        mp = APS.tile([128, 512], F32, tag=f"m{sl}", name=f"mp{sl}")
                        m_ps.append(mp)
                        nc.tensor.matmul(
                            mp[:], kt_all[:, h, :], qt[:, cs], start=True, stop=True,
                        )
                        nc.scalar.activation(
                            expt[:, cs], mp[:], AF.Exp,
                            bias=mbq[:, h : h + 1], scale=SCALE,
                        )
                    for sl in range(NSL):
                        cs = slice(sl * 512, (sl + 1) * 512)
                        den_ps = APS.tile([128, 512], F32, tag=f"d{sl}", name=f"dp{sl}")
                        # ones16 = 1/16 -> den_ps = den/16; rcp = 16/den
                        nc.tensor.matmul(
                            den_ps[:], ones16[:], expt[:, cs], start=True, stop=True
                        )
                        rcp = RCP.tile([128, 512], F32, tag="rcp")
                        nc.vector.reciprocal(rcp[:], den_ps[:])
                        # probs8 = expt * 16/den (fp8, scaled into normal range)
                        nc.gpsimd.tensor_tensor(
                            pr8[:, cs], expt[:, cs], rcp[:], ALU.mult
                        )
                        nc.tensor.matmul(
                            m_ps[sl][:], v_all[:, h, :], pr8[:, cs],
                            start=True, stop=True,
                        )
                        # att8 = (16*att)/16
                        nc.scalar.activation(
                            att8[:, h, cs], m_ps[sl][:], AF.Copy, scale=1.0 / 16.0
                        )

            # ---------- phase 4: out-proj + residual ----------
            with (
                tc.tile_pool(name="wo", bufs=2) as WO,
                tc.tile_pool(name="ops", bufs=2, space="PSUM") as OPS,
                tc.tile_pool(name="xrs", bufs=2) as XRS,
                tc.tile_pool(name="osb", bufs=3) as OSB,
            ):
                for nt in range(NT):
                    wo = WO.tile([128, KT, 128], F8, tag="wo")
                    nc.sync.dma_start(
                        wo[:],
                        wot_in[:, nt * 128 : (nt + 1) * 128].rearrange(
                            "(ht p) n -> p ht n", p=128
                        ),
                    )
                    xr = XRS.tile([128, SC], F32, tag="xr")
                    nc.sync.dma_start(xr[:], xtf_in[nt * 128 : (nt + 1) * 128, :])
                    o_ps = [OPS.tile([128, 512], F32, tag=f"o{sl}", name=f"op{sl}") for sl in range(NSL)]
                    for sl in range(NSL):
                        cs = slice(sl * 512, (sl + 1) * 512)
                        for hp in range(KP):
                            nc.tensor.matmul(
                                o_ps[sl][:], wo[:, 2 * hp : 2 * hp + 2, :],
                                att8[:, 2 * hp : 2 * hp + 2, cs],
                                start=(hp == 0), stop=(hp == KP - 1), perf_mode=DR,
                            )
                    for sl in range(NSL):
                        cs = slice(sl * 512, (sl + 1) * 512)
                        osb = OSB.tile([128, 512], F32, tag="osb")
                        nc.vector.scalar_tensor_tensor(
                            osb[:], o_ps[sl][:], 1.0 / WS, xr[:, cs],
                            ALU.mult, ALU.add,
                        )
                        nc.sync.dma_start(
                            out_t[nt * 128 : (nt + 1) * 128, cs], osb[:]
                        )

    if split_waits:
        _split_waits(nc)
    return nc


_NC_CACHE = None
_LAST_IN_MAPS = None


def prep_inputs(
    hidden_states, memory_keys, memory_values, attention_mask, Wq, Wout,
    ln_gamma, ln_beta,
):
    f32 = np.float32
    bf16 = ml_dtypes.bfloat16
    f8 = ml_dtypes.float8_e4m3
    x = np.asarray(hidden_states, dtype=f32).reshape(B * S, HID)
    gamma = np.asarray(ln_gamma, dtype=f32)
    beta = np.asarray(ln_beta, dtype=f32)
    Wq = np.asarray(Wq, dtype=f32)
    Wout = np.asarray(Wout, dtype=f32)

    wqt8 = np.ascontiguousarray((Wq * gamma[None, :]).T * WS).astype(f8)
    wot8 = np.ascontiguousarray(Wout.T * WS).astype(f8)
    csn = (-wqt8.astype(f32).sum(0, keepdims=True)).astype(bf16)  # [1, HID]
    bq = (Wq @ beta).astype(f32)                                  # [HID]

    kts, vs, mbqs = [], [], []
    for b in range(B):
        kb = np.asarray(memory_keys[b], dtype=f32).reshape(SLOTS, HEADS, DH)
        vb = np.asarray(memory_values[b], dtype=f32).reshape(SLOTS, HEADS, DH)
        kts.append(np.ascontiguousarray(kb.transpose(1, 2, 0)).astype(bf16))
        vs.append(np.ascontiguousarray(vb.transpose(1, 0, 2)).astype(bf16))
        m = np.asarray(attention_mask[b]).astype(bool)
        mb = np.where(m, 0.0, -1e30).astype(f32)
        kbq = np.einsum("thd,hd->th", kb, bq.reshape(HEADS, DH)) * SCALE
        mbqs.append((mb[:, None] + kbq).astype(f32))              # [SLOTS, HEADS]

    in_maps = []
    for c in range(NC_):
        rows = slice(c * SC, (c + 1) * SC)
        xt = np.ascontiguousarray(x[rows].T)  # [HID, SC] f32
        b = (c * SC) // S
        in_maps.append(
            dict(
                xt8=xt.astype(f8),
                xtf=xt,
                wqt=wqt8,
                wot=wot8,
                csn=csn,
                ktt=kts[b],
                vv=vs[b],
                mbq=mbqs[b],
            )
        )
    return in_maps


def kernel(
    hidden_states, memory_keys, memory_values, attention_mask, Wq, Wout,
    ln_gamma, ln_beta,
):
    global _NC_CACHE
    if _NC_CACHE is None:
        _NC_CACHE = build_nc()
    nc = _NC_CACHE

    in_maps = prep_inputs(
        hidden_states, memory_keys, memory_values, attention_mask, Wq, Wout,
        ln_gamma, ln_beta,
    )
    global _LAST_IN_MAPS
    _LAST_IN_MAPS = in_maps
    from concourse import bass2jax

    results = bass2jax.run_bass_via_pjrt(nc, in_maps, n_cores=NC_)

    f32 = np.float32
    out = np.empty((B * S, HID), dtype=f32)
    for c in range(NC_):
        out[c * SC : (c + 1) * SC] = results[c]["outt"].T
    return out.reshape(B, S, HID)
